# revision 1
# baseline (speedup 1.0000x reference)
"""TRN2 Bass kernel for nn_ACoef: out[b] = sum_ij coef[i,j] * traces[b,i,j] / norm[i,j]
where traces[b,i,j] = sum_n diag(x_b^(i+2))[n]^(j+1), x: [B=1024, N=224, N] fp32.

Data-parallel over 8 NeuronCores, 128 matrices each. Per matrix, with z = x^T
(powers of z have the same diagonals as powers of x):

  chain (TensorE): Q2 = z@z, Q3 = z@Q2, Q4 = z@Q3    [lhsT = x natural!]
  diagonals (VectorE fused mult+rowsum, fp32-exact operands):
      d2 = rowsum(Q2 * eyemask)          = diag(x^2)
      d3 = rowsum(Q2 * x)                = diag(x^3)   [z^3 diag: sum_k Q2[n,k]*x[n,k]]
      d4 = rowsum(Q3 * x)                = diag(x^4)
      d5 = rowsum(Q4 * x)                = diag(x^5)
  tail: v = d/N^2;  out[b] = sum_ij w[i,j] * sum_n v_i[n]^(j+1),
        w[i,j] = coef[i,j]/(N^2)^(i+1)   (rescaled to avoid fp32 denormals)

Numerics: matmul-operand rounding (bf16/f32r) adds only incoherent noise
(validated ~4e-4 max rel vs the fp32 reference); the elementwise product
operands (Q* from PSUM / SBUF fp32 copies, x, eyemask) stay fp32-exact.
"""
import os
import sys
import types
import numpy as np

import concourse.bass as bass
import concourse.bacc as bacc
import concourse.mybir as mybir
from concourse import tile
from concourse.bass_utils import run_bass_kernel_spmd

dt = mybir.dt
F32 = dt.float32

B, N = 1024, 224
NCORES = 8
PA, PB = 128, N - 128          # row-tile partition sizes (128 + 96)
ROWS, COLS = 4, 4
CHAIN = os.environ.get("ACOEF_CHAIN", "f32r")  # f32r | f32 | bf16
DBG = set(filter(None, os.environ.get("ACOEF_DBG", "").split(",")))
MUL = mybir.AluOpType.mult
ADD = mybir.AluOpType.add


def _install_ntff_shim():
    """Register the axon NTFF profile hook the stub `antenv` package lacks."""
    try:
        import antenv
        if "antenv.axon_hooks" in sys.modules:
            return
        mod = types.ModuleType("antenv.axon_hooks")
        mod._hook = None
        mod.set_axon_ntff_profile_hook = lambda h: setattr(mod, "_hook", h)
        mod.get_axon_ntff_profile_hook = lambda: mod._hook
        sys.modules["antenv.axon_hooks"] = mod
        antenv.axon_hooks = mod
        from trn_agent_boot.trn_boot import _ntff_profile_via_ctypes
        mod._hook = _ntff_profile_via_ctypes("/opt/axon/libaxon_pjrt.so")
    except Exception:
        pass


def build_program(C):
    """Per-core Bass program over C matrices."""
    if CHAIN == "f32r":
        CH, W = dt.float32r, 256     # pad moving free-dim to 256 (full-rate f32r)
    elif CHAIN == "bf16":
        CH, W = dt.bfloat16, N
    else:
        CH, W = F32, N

    nc = bacc.Bacc("TRN2", target_bir_lowering=False, debug=False)
    x_d = nc.dram_tensor("x", [C, N, N], CH, kind="ExternalInput").ap()
    eye_d = nc.dram_tensor("eye", [128, 128], F32, kind="ExternalInput").ap()
    ones_d = nc.dram_tensor("ones", [128, 1], F32, kind="ExternalInput").ap()
    w_d = nc.dram_tensor("wbig", [128, 16 * C], F32, kind="ExternalInput").ap()
    out_d = nc.dram_tensor("out", [C], F32, kind="ExternalOutput").ap()
    if "dumpd" in DBG:
        da_d = nc.dram_tensor("dda", [PA, 4 * C], F32, kind="ExternalOutput").ap()
        db_d = nc.dram_tensor("ddb", [PB, 4 * C], F32, kind="ExternalOutput").ap()

    with tile.TileContext(nc) as tc:
        with (
            tc.tile_pool(name="const", bufs=1) as constp,
            tc.tile_pool(name="dbig", bufs=1) as dbigp,
            tc.tile_pool(name="tailp", bufs=1) as tailp,
        ):
            eye = constp.tile([128, 128], F32, tag="eye")
            ones = constp.tile([128, 1], F32, tag="ones")
            wbig = constp.tile([128, 16 * C], F32, tag="wbig")
            nc.sync.dma_start(eye[:], eye_d)
            nc.sync.dma_start(ones[:], ones_d)
            nc.sync.dma_start(wbig[:], w_d)

            Da = dbigp.tile([PA, 4 * C], F32, tag="Da")
            Db = dbigp.tile([PB, 4 * C], F32, tag="Db")

            SCALE = 1.0 / float(N * N)

            with (
                tc.tile_pool(name="xp", bufs=3) as xp,
                tc.tile_pool(name="zp", bufs=3) as zp,
                tc.tile_pool(name="qp", bufs=3) as qp,
                tc.tile_pool(name="scr", bufs=2) as scr,
                tc.tile_pool(name="ps", bufs=1, space="PSUM") as ps,
                tc.tile_pool(name="pst", bufs=1, space="PSUM") as pst,
            ):
                for m in range(C):
                    # ---- load x (natural); CH-typed, exact fp32 bytes ----
                    xA = xp.tile([PA, N], CH, tag="xA")
                    xB = xp.tile([PB, N], CH, tag="xB")
                    nc.sync.dma_start(xA[:], x_d[m, 0:PA, :])
                    nc.sync.dma_start(xB[:], x_d[m, PA:N, :])
                    lA, lB = xA, xB
                    pxA, pxB = xA.bitcast(F32), xB.bitcast(F32)

                    # ---- z = x^T in chain dtype ----
                    zA = zp.tile([PA, W], CH, tag="zA")
                    zB = zp.tile([PB, W], CH, tag="zB")
                    # PE transpose (fp32-exact) -> PSUM; feeds the exact d2
                    # product AND (via rounding ACT copy) the chain moving side
                    ztA = pst.tile([PA, N], F32, tag="ztA")
                    ztB = pst.tile([PB, N], F32, tag="ztB")
                    fA, fB = pxA, pxB
                    nc.tensor.matmul(ztA[:, 0:128], fA[:, 0:128], eye[:],
                                     is_transpose=True, start=True, stop=False)
                    nc.tensor.matmul(ztA[:, 128:N], fB[:, 0:128],
                                     eye[0:PB, 0:PB],
                                     is_transpose=True, start=False, stop=True)
                    nc.tensor.matmul(ztB[:, 0:128], fA[:, 128:N], eye[:],
                                     is_transpose=True, start=True, stop=False)
                    nc.tensor.matmul(ztB[:, 128:N], fB[:, 128:N],
                                     eye[0:PB, 0:PB],
                                     is_transpose=True, start=False, stop=True)
                    nc.scalar.copy(zA[:, 0:N], ztA[:])
                    nc.scalar.copy(zB[:, 0:N], ztB[:])
                    if W > N:
                        # pad cols: junk values, only to initialize the
                        # full moving-operand width (outputs unused)
                        nc.scalar.copy(zA[:, N:W], ztA[:, 0:W - N])
                        nc.scalar.copy(zB[:, N:W], ztB[:, 0:W - N])

                    def chain_mm(qA, qB, rA, rB):
                        # Q = z @ r, z = x^T: out[mt,n] = sum_k x[k,mt]*r[k,n]
                        nc.tensor.matmul(qA[:], lA[:, 0:128], rA[:],
                                         start=True, stop=False)
                        nc.tensor.matmul(qA[:], lB[:, 0:128], rB[:],
                                         start=False, stop=True)
                        nc.tensor.matmul(qB[:], lA[:, 128:N], rA[:],
                                         start=True, stop=False)
                        nc.tensor.matmul(qB[:], lB[:, 128:N], rB[:],
                                         start=False, stop=True)

                    def copy_q(qA, qB, tag):
                        cA = qp.tile([PA, W], CH, tag=tag + "A")
                        cB = qp.tile([PB, W], CH, tag=tag + "B")
                        nc.scalar.copy(cA[:], qA[:])
                        nc.scalar.copy(cB[:], qB[:])
                        return cA, cB

                    zB_mm = zB[:]

                    q2_pA = ps.tile([PA, W], F32, tag="q2A")
                    q2_pB = ps.tile([PB, W], F32, tag="q2B")
                    chain_mm(q2_pA, q2_pB, zA[:], zB_mm)
                    q2A, q2B = copy_q(q2_pA, q2_pB, "q2")

                    q3_pA = ps.tile([PA, W], F32, tag="q3A")
                    q3_pB = ps.tile([PB, W], F32, tag="q3B")
                    chain_mm(q3_pA, q3_pB, q2A[:], q2B[:])
                    q3A, q3B = copy_q(q3_pA, q3_pB, "q3")

                    q4_pA = ps.tile([PA, W], F32, tag="q4A")
                    q4_pB = ps.tile([PB, W], F32, tag="q4B")
                    chain_mm(q4_pA, q4_pB, q3A[:], q3B[:])

                    # ---- products -> Dbig[:, 4m+i], scaled by 1/N^2 ----
                    def prod(i, in0A, in0B, in1A, in1B):
                        oA = scr.tile([PA, N], F32, tag="scrA")
                        oB = scr.tile([PB, N], F32, tag="scrB")
                        col = 4 * m + i
                        if "nottr" in DBG:
                            nc.vector.tensor_copy(oA[:], in0A)
                            nc.vector.tensor_copy(oB[:], in0B)
                            nc.vector.tensor_copy(Da[:, col:col + 1], oA[:, 0:1])
                            nc.vector.tensor_copy(Db[:, col:col + 1], oB[:, 0:1])
                            return
                        nc.vector.scalar_tensor_tensor(
                            oA[:], in0A, SCALE, in1A, MUL, MUL,
                            accum_out=Da[:, col:col + 1])
                        nc.vector.scalar_tensor_tensor(
                            oB[:], in0B, SCALE, in1B, MUL, MUL,
                            accum_out=Db[:, col:col + 1])

                    # products must read fp32-exact values -> PSUM/raw-x
                    prod(0, ztA[:], ztB[:], pxA[:], pxB[:])
                    prod(1, q2_pA[:, 0:N], q2_pB[:, 0:N], pxA[:], pxB[:])
                    prod(2, q3_pA[:, 0:N], q3_pB[:, 0:N], pxA[:], pxB[:])
                    prod(3, q4_pA[:, 0:N], q4_pB[:, 0:N], pxA[:], pxB[:])

            if "dumpd" in DBG:
                nc.sync.dma_start(da_d, Da[:])
                nc.sync.dma_start(db_d, Db[:])
            # ================= tail =================
            if "notail" in DBG:
                out_sb = tailp.tile([1, C], F32, tag="outsb")
                nc.vector.tensor_copy(out_sb[:], Da[0:1, 0:C])
                nc.sync.dma_start(out_d.rearrange("(o c) -> o c", o=1), out_sb[:])
            elif True:
              with tc.tile_pool(name="pso", bufs=1, space="PSUM") as pso:
                  outp = pso.tile([1, C], F32, tag="outp")
                  for half, (D, P) in enumerate([(Da, PA), (Db, PB)]):
                      C4 = 4 * C
                      T = tailp.tile([P, 16 * C], F32, tag=f"T{half}")
                      TW = tailp.tile([P, 16 * C], F32, tag=f"TW{half}")
                      nc.vector.tensor_copy(T[:, 0:C4], D[:])
                      nc.vector.tensor_tensor(T[:, C4:2 * C4], D[:], D[:], MUL)
                      nc.vector.tensor_tensor(T[:, 2 * C4:3 * C4],
                                              T[:, C4:2 * C4], D[:], MUL)
                      nc.vector.tensor_tensor(T[:, 3 * C4:4 * C4],
                                              T[:, C4:2 * C4], T[:, C4:2 * C4], MUL)
                      nc.vector.tensor_tensor(TW[:], T[:], wbig[0:P, :], MUL)
                      R = tailp.tile([P, C], F32, tag=f"R{half}")
                      tw4 = TW[:].rearrange("p (j m i) -> p m j i", j=4, i=4)
                      nc.vector.tensor_reduce(R[:], tw4, mybir.AxisListType.XY, ADD)
                      nc.tensor.matmul(outp[:], ones[0:P, :], R[:],
                                       start=(half == 0), stop=(half == 1))
                  out_sb = tailp.tile([1, C], F32, tag="outsb")
                  nc.vector.tensor_copy(out_sb[:], outp[:])
                  nc.sync.dma_start(out_d.rearrange("(o c) -> o c", o=1), out_sb[:])

    nc.compile()
    return nc


_PROGRAM_CACHE = {}


def _get_program(C):
    if C not in _PROGRAM_CACHE:
        _PROGRAM_CACHE[C] = build_program(C)
    return _PROGRAM_CACHE[C]


def make_host_inputs(coef, C):
    eye = np.eye(128, dtype=np.float32)
    ones = np.ones((128, 1), dtype=np.float32)
    # w[i, j] = coef[i, j] / (N^2)^(i+1)  (d's pre-scaled by 1/N^2 on-chip)
    ii = np.arange(ROWS, dtype=np.float64)[:, None]
    w = np.asarray(coef, np.float64) / (float(N * N) ** (ii + 1.0))
    # wbig[p, j*4C + 4m + i] = w[i, j]
    wrow = np.zeros((16 * C,), np.float64)
    for j in range(COLS):
        wrow[j * 4 * C:(j + 1) * 4 * C] = np.tile(w[:, j], C)
    wbig = np.broadcast_to(wrow, (128, 16 * C)).astype(np.float32).copy()
    return eye, ones, wbig


def _in_maps(x, coef, C):
    eye, ones, wbig = make_host_inputs(coef, C)
    return [{"x": x[c * C:(c + 1) * C], "eye": eye, "ones": ones, "wbig": wbig}
            for c in range(NCORES)]


def kernel(x, coef):
    x = np.ascontiguousarray(np.asarray(x, np.float32))
    coef = np.asarray(coef, np.float32)
    C = x.shape[0] // NCORES
    nc = _get_program(C)
    res = run_bass_kernel_spmd(nc, _in_maps(x, coef, C),
                               core_ids=list(range(NCORES)))
    return np.concatenate([res.results[c]["out"] for c in range(NCORES)])


def kernel_traced(x, coef):
    """Like kernel() but also returns exec_time_ns (NTFF profile)."""
    _install_ntff_shim()
    x = np.ascontiguousarray(np.asarray(x, np.float32))
    coef = np.asarray(coef, np.float32)
    C = x.shape[0] // NCORES
    nc = _get_program(C)
    maps = _in_maps(x, coef, C)
    res = run_bass_kernel_spmd(nc, maps, core_ids=list(range(NCORES)))
    out = np.concatenate([res.results[c]["out"] for c in range(NCORES)])
    exec_ns = None
    try:
        res2 = run_bass_kernel_spmd(nc, maps, core_ids=list(range(NCORES)),
                                    trace=True)
        exec_ns = res2.exec_time_ns
    except Exception as e:
        print(f"trace failed: {type(e).__name__}: {str(e)[:200]}")
    return out, exec_ns

